# revision 8
# baseline (speedup 1.0000x reference)
"""Trainium2 Bass kernel for nn_Head (single-head causal attention, T=8).

Pure data parallel over 8 NeuronCores: per core x [4096, 8, 384] -> out
[4096, 8, 64]. The host marshals x into transposed bf16 layout
xT [ns, 128c, 3, 512tok] so the device streams contiguous tiles and never
transposes activations on-chip.

Per 512-token supertile (tokens on partitions in 4 groups of 128),
software-pipelined 3 deep on the PE (projections for s, scores for s-1,
output for s-2) so every PE operand is ready >= 1 iteration early and the
tensor engine never idles (keeps the 2.4 GHz p-state):
  1. DMA xT chunk [128, 3, 512] bf16 (contiguous)
  2. QK^T = [Wq|Wk]^T @ x^T -> qk [128(2h), 512] (3 fused MMs, W stationary)
  3. shuffle K^T (partitions 64:128) down to base 0 via SBUF->SBUF DMA
  4. V natural [tok, 64] = x @ Wv via lhsT=xT slices (12 MMs), interleaved
     with the s-1 score MMs and s-2 output MMs
  5. S^T[k,q] per group: PSUM preloaded with +800*mask via a matmul
     (lhsT=mask^T, rhs=800*I), then lhsT=K^T rhs=Q^T accumulates on top;
     exp(x*SCALE - 800*SCALE) on ACT zeroes forbidden entries -- no
     separate mask op on any vector engine
  6. out[q, 0:65] = S~ @ [V|1] via lhsT=S~^T (4 MMs); col 64 = softmax denom
  7. normalize: reciprocal + scaled copy (DVE), bf16 out
  8. DMA out bf16, partition-major, batched 2 supertiles per descriptor

bf16 on matmul paths with f32 PSUM accumulation: rel error ~3e-3.
"""

import numpy as np
import ml_dtypes

import concourse.bass as bass
import concourse.mybir as mybir
from concourse import bacc
from concourse.tile import TileContext
from concourse.bass_utils import run_bass_kernel_spmd

N_CORES = 8
B_FULL = 32768
T = 8
C = 384
H = 64

BP = B_FULL // N_CORES       # batch rows per core
TOK = BP * T                 # tokens per core
ST = 512                     # tokens per supertile
G = ST // 128                # 128-token groups per supertile
NCH = C // 128               # contraction chunks
SCALE = float(C) ** -0.5
NV = 4                       # persistent [V|1] buffers (pipeline depth 3)
MB = 800.0                   # mask bias magnitude (exact in bf16)

BF16 = mybir.dt.bfloat16
F32 = mybir.dt.float32
AF = mybir.ActivationFunctionType

_nc_cache = {}


def _build_nc(ns: int):
    """Build the Bass module for `ns` supertiles per core."""
    assert ns % 2 == 0
    nc = bacc.Bacc("TRN2", target_bir_lowering=False, debug=False)

    xtd = nc.dram_tensor("xt", [ns, 128, NCH, ST], BF16, kind="ExternalInput")
    wqkd = nc.dram_tensor("wqk", [128, NCH, 2 * H], BF16, kind="ExternalInput")
    wvd = nc.dram_tensor("wv", [128, NCH, H], BF16, kind="ExternalInput")
    masktt_d = nc.dram_tensor("masktt", [128, 128], BF16, kind="ExternalInput")
    eye_d = nc.dram_tensor("eyemb", [128, 128], BF16, kind="ExternalInput")
    od = nc.dram_tensor("out", [ns // 2, 128, 2, G, H], BF16,
                        kind="ExternalOutput")

    with TileContext(nc) as tc:
        with (
            tc.tile_pool(name="const", bufs=1) as cpool,
            tc.tile_pool(name="xt", bufs=4) as xtpool,
            tc.tile_pool(name="qk", bufs=3) as qkpool,
            tc.tile_pool(name="sm", bufs=3) as smpool,
            tc.tile_pool(name="oo", bufs=3) as opool,
            tc.tile_pool(name="ps_qk", bufs=2, space="PSUM") as pqk,
            tc.tile_pool(name="ps_st", bufs=2, space="PSUM") as pst,
            tc.tile_pool(name="ps_v", bufs=2, space="PSUM") as pv,
            tc.tile_pool(name="ps_o", bufs=2, space="PSUM") as po,
        ):
            masktt = cpool.tile([128, 128], BF16)
            nc.sync.dma_start(masktt, masktt_d[:, :])
            eyemb = cpool.tile([128, 128], BF16)
            nc.sync.dma_start(eyemb, eye_d[:, :])
            wqk = cpool.tile([128, NCH, 2 * H], BF16)
            nc.sync.dma_start(wqk, wqkd[:, :, :])
            wv = cpool.tile([128, NCH, H], BF16)
            nc.sync.dma_start(wv, wvd[:, :, :])
            # persistent [V|1] tiles: ones column written once, V columns
            # rewritten every NV iterations (WAR tracked by the framework)
            v_tiles = [
                cpool.tile([128, G, H + 1], BF16, name=f"v{i}", tag=f"v{i}")
                for i in range(NV)
            ]
            for vt in v_tiles:
                nc.gpsimd.memset(vt[:, :, H:H + 1], 1.0)
            ebias = cpool.tile([128, 1], F32)
            nc.gpsimd.memset(ebias, -MB * SCALE)

            # per-stage state carried across pipeline iterations
            xt_sb = [None] * ns
            qk_sb = [None] * ns
            kt_sb = [None] * ns
            sm_sb = [None] * ns
            st_ps = [None] * ns
            v_ps = [None] * ns
            o_ps = [None] * ns
            o_pair = [None] * (ns // 2)

            for it in range(ns + 3):
                s = it          # projection stage
                s1 = it - 1     # scores stage
                s2 = it - 2     # output stage
                s3 = it - 3     # store stage

                if 0 <= s3 < ns and s3 % 2 == 1:
                    # 8. store 2 supertiles bf16, partition-major (1KB/part).
                    # Lagged so its dependency is complete: never stalls the
                    # sync prefetch queue.
                    nc.sync.dma_start(od[s3 // 2], o_pair[s3 // 2])

                if s < ns:
                    # 1. load xT (already transposed + bf16 on host); sync
                    # queue carries only cheap issues so it prefetches ahead
                    xt_sb[s] = xtpool.tile([128, NCH, ST], BF16,
                                           name="xt", tag="xt")
                    nc.sync.dma_start(xt_sb[s], xtd[s])

                    # 2. fused Q^T|K^T: one MM per chunk, 128 PE columns
                    qk_ps = pqk.tile([128, ST], F32, name="qkps", tag="qkps")
                    for j in range(NCH):
                        nc.tensor.matmul(
                            qk_ps,
                            lhsT=wqk[:, j, :],
                            rhs=xt_sb[s][:, j, :],
                            start=(j == 0),
                            stop=(j == NCH - 1),
                        )

                # 4/5/6 interleaved on PE: V(s) hides its weight loads
                # under the longer score/output streams of older stages
                for g in range(G):
                    if s < ns:
                        if g == 0:
                            v_ps[s] = pv.tile([128, G, H], F32,
                                              name="vps", tag="vps")
                        for j in range(NCH):
                            nc.tensor.matmul(
                                v_ps[s][:, g, :],
                                lhsT=xt_sb[s][:, j, g * 128:(g + 1) * 128],
                                rhs=wv[:, j, :],
                                start=(j == 0),
                                stop=(j == NCH - 1),
                            )
                    if 0 <= s1 < ns:
                        if g == 0:
                            st_ps[s1] = pst.tile([128, G, 128], F32,
                                                 name="stps", tag="stps")
                        # mask bias preload: st[s,q] = MB*mask[s,q]
                        nc.tensor.matmul(
                            st_ps[s1][:, g, :],
                            lhsT=masktt,
                            rhs=eyemb,
                            start=True,
                            stop=False,
                        )
                        nc.tensor.matmul(
                            st_ps[s1][:, g, :],
                            lhsT=kt_sb[s1][:, g * 128:(g + 1) * 128],
                            rhs=qk_sb[s1][0:H, g * 128:(g + 1) * 128],
                            start=False,
                            stop=True,
                        )
                    if 0 <= s2 < ns:
                        if g == 0:
                            o_ps[s2] = po.tile([128, G, H + 1], F32,
                                               name="ops", tag="ops")
                        nc.tensor.matmul(
                            o_ps[s2][:, g, :],
                            lhsT=sm_sb[s2][:, g, :],
                            rhs=v_tiles[s2 % NV][:, g, :],
                            start=True,
                            stop=True,
                        )

                if s < ns:
                    # 2b. PSUM->SBUF, then 3. shuffle K^T down to base 0
                    qk_sb[s] = qkpool.tile([128, ST], BF16, name="qk", tag="qk")
                    nc.vector.tensor_copy(qk_sb[s], qk_ps)
                    kt_sb[s] = qkpool.tile([64, ST], BF16, name="kt", tag="kt")
                    nc.gpsimd.dma_start(kt_sb[s], qk_sb[s][H:2 * H, :])

                if 0 <= s1 < ns:
                    # 5b. exp((scores + MB*mask)*SCALE - MB*SCALE): allowed
                    # entries exact, forbidden -> exp(-40.8) ~ 0
                    sm_sb[s1] = smpool.tile([128, G, 128], BF16,
                                            name="sm", tag="sm")
                    nc.scalar.activation(sm_sb[s1], st_ps[s1], AF.Exp,
                                         scale=SCALE, bias=ebias[:, 0:1])

                if s < ns:
                    # 4b. V PSUM->SBUF into the persistent [V|1] tile
                    nc.scalar.copy(v_tiles[s % NV][:, :, 0:H], v_ps[s])

                if 0 <= s2 < ns:
                    # 7. normalize (recip broadcast along heads), bf16 out
                    if s2 % 2 == 0:
                        o_pair[s2 // 2] = opool.tile([128, 2, G, H], BF16,
                                                     name="o", tag="o")
                    recip = opool.tile([128, G], F32, name="recip", tag="recip")
                    nc.vector.reciprocal(recip, o_ps[s2][:, :, H])
                    nc.vector.tensor_mul(
                        o_pair[s2 // 2][:, s2 % 2, :, :],
                        o_ps[s2][:, :, 0:H],
                        recip[:, :, None].to_broadcast([128, G, H]),
                    )

    nc.finalize()
    return nc


def _consts():
    bf = ml_dtypes.bfloat16
    maskt = np.kron(
        np.eye(128 // T, dtype=np.float32),
        np.triu(np.ones((T, T), dtype=np.float32)),
    )
    masktt = np.ascontiguousarray(maskt.T).astype(bf)
    eyemb = (MB * np.eye(128, dtype=np.float32)).astype(bf)
    return masktt, eyemb


def _prepare(x, Wq, Wk, Wv):
    """Returns (nc, in_maps) for the full-size problem."""
    assert x.shape == (B_FULL, T, C), x.shape
    ns = TOK // ST
    if ns not in _nc_cache:
        _nc_cache[ns] = _build_nc(ns)
    nc = _nc_cache[ns]

    bf = ml_dtypes.bfloat16
    wqk_full = np.concatenate([Wq, Wk], axis=1)  # [C, 2H]
    wqk_h = np.ascontiguousarray(
        wqk_full.reshape(NCH, 128, 2 * H).transpose(1, 0, 2)
    ).astype(bf)
    wv_h = np.ascontiguousarray(
        Wv.reshape(NCH, 128, H).transpose(1, 0, 2)
    ).astype(bf)
    masktt, eyemb = _consts()

    # host-side marshalling: bf16 cast + transpose to [ns, 128c, NCH, ST]
    xb = x.reshape(N_CORES, TOK // ST, ST, NCH, 128).astype(bf)
    in_maps = []
    for c in range(N_CORES):
        xs = np.ascontiguousarray(xb[c].transpose(0, 3, 2, 1))
        in_maps.append({
            "xt": xs, "wqk": wqk_h, "wv": wv_h,
            "masktt": masktt, "eyemb": eyemb,
        })
    return nc, in_maps


def _gather(results):
    ns = TOK // ST
    outs = [
        np.asarray(r["out"])
        .reshape(ns // 2, 128, 2, G, H)
        .transpose(0, 2, 3, 1, 4)
        .reshape(BP, T, H)
        .astype(np.float32)
        for r in results
    ]
    return np.concatenate(outs, axis=0)


def kernel(x, Wq, Wk, Wv):
    nc, in_maps = _prepare(x, Wq, Wk, Wv)
    res = run_bass_kernel_spmd(nc, in_maps, core_ids=list(range(N_CORES)))
    return _gather(res.results)


# revision 9
# speedup vs baseline: 1.1995x; 1.1995x over previous
"""Trainium2 Bass kernel for nn_Head (single-head causal attention, T=8).

Pure data parallel over 8 NeuronCores: per core x [4096, 8, 384] -> out
[4096, 8, 64]. The host marshals x into transposed bf16 layout
xT [ns, 128c, 3, 512tok] so the device streams contiguous tiles and never
transposes activations on-chip.

Per 512-token supertile (tokens on partitions in 4 groups of 128),
software-pipelined 3 deep on the PE (projections for s, scores for s-1,
output for s-2) so every PE operand is ready >= 1 iteration early and the
tensor engine never idles (keeps the 2.4 GHz p-state):
  1. DMA xT chunk [128, 3, 512] bf16 (contiguous)
  2. QK^T = [Wq|Wk]^T @ x^T -> qk [128(2h), 512] (3 fused MMs, W stationary)
  3. shuffle K^T (partitions 64:128) down to base 0 via SBUF->SBUF DMA
  4. V natural [tok, 64] = x @ Wv via lhsT=xT slices (12 MMs), interleaved
     with the s-1 score MMs and s-2 output MMs
  5. S^T[k,q] per group: PSUM preloaded with +800*mask via a matmul
     (lhsT=mask^T, rhs=800*I), then lhsT=K^T rhs=Q^T accumulates on top;
     exp(x*SCALE - 800*SCALE) on ACT zeroes forbidden entries -- no
     separate mask op on any vector engine
  6. out[q, 0:65] = S~ @ [V|1] via lhsT=S~^T (4 MMs); col 64 = softmax denom
  7. normalize: reciprocal + scaled copy (DVE), bf16 out
  8. DMA out bf16, partition-major, batched 2 supertiles per descriptor

bf16 on matmul paths with f32 PSUM accumulation: rel error ~3e-3.
"""

import numpy as np
import ml_dtypes

import concourse.bass as bass
import concourse.mybir as mybir
from concourse import bacc
from concourse.tile import TileContext
from concourse.bass_utils import run_bass_kernel_spmd

N_CORES = 8
B_FULL = 32768
T = 8
C = 384
H = 64

BP = B_FULL // N_CORES       # batch rows per core
TOK = BP * T                 # tokens per core
ST = 512                     # tokens per supertile
G = ST // 128                # 128-token groups per supertile
NCH = C // 128               # contraction chunks
SCALE = float(C) ** -0.5
NV = 4                       # persistent [V|1] buffers (pipeline depth 3)
MB = 800.0                   # mask bias magnitude (exact in bf16)

BF16 = mybir.dt.bfloat16
F32 = mybir.dt.float32
AF = mybir.ActivationFunctionType

_nc_cache = {}


def _build_nc(ns: int):
    """Build the Bass module for `ns` supertiles per core."""
    assert ns % 2 == 0
    nc = bacc.Bacc("TRN2", target_bir_lowering=False, debug=False)

    xtd = nc.dram_tensor("xt", [ns, 128, NCH, ST], BF16, kind="ExternalInput")
    wqkd = nc.dram_tensor("wqk", [128, NCH, 2 * H], BF16, kind="ExternalInput")
    wvd = nc.dram_tensor("wv", [128, NCH, H], BF16, kind="ExternalInput")
    masktt_d = nc.dram_tensor("masktt", [128, 128], BF16, kind="ExternalInput")
    eye_d = nc.dram_tensor("eyemb", [128, 128], BF16, kind="ExternalInput")
    od = nc.dram_tensor("out", [ns // 2, 128, 2, G, H], BF16,
                        kind="ExternalOutput")

    with TileContext(nc) as tc:
        with (
            tc.tile_pool(name="const", bufs=1) as cpool,
            tc.tile_pool(name="xt", bufs=4) as xtpool,
            tc.tile_pool(name="qk", bufs=3) as qkpool,
            tc.tile_pool(name="sm", bufs=3) as smpool,
            tc.tile_pool(name="oo", bufs=3) as opool,
            tc.tile_pool(name="ps_qk", bufs=2, space="PSUM") as pqk,
            tc.tile_pool(name="ps_st", bufs=2, space="PSUM") as pst,
            tc.tile_pool(name="ps_v", bufs=2, space="PSUM") as pv,
            tc.tile_pool(name="ps_o", bufs=2, space="PSUM") as po,
        ):
            masktt = cpool.tile([128, 128], BF16)
            nc.sync.dma_start(masktt, masktt_d[:, :])
            eyemb = cpool.tile([128, 128], BF16)
            nc.sync.dma_start(eyemb, eye_d[:, :])
            wqk = cpool.tile([128, NCH, 2 * H], BF16)
            nc.sync.dma_start(wqk, wqkd[:, :, :])
            wv = cpool.tile([128, NCH, H], BF16)
            nc.sync.dma_start(wv, wvd[:, :, :])
            # persistent [V|1] tiles: ones column written once, V columns
            # rewritten every NV iterations (WAR tracked by the framework)
            v_tiles = [
                cpool.tile([128, G, H + 1], BF16, name=f"v{i}", tag=f"v{i}")
                for i in range(NV)
            ]
            for vt in v_tiles:
                nc.gpsimd.memset(vt[:, :, H:H + 1], 1.0)
            ebias = cpool.tile([128, 1], F32)
            nc.gpsimd.memset(ebias, -MB * SCALE)
            # persistent K^T tiles, full 128 partitions so the score MM keeps
            # the (128,128) PE tile config (a 64-row MM costs a ~125ns
            # reconfig); rows 64:128 stay zero to kill the K^T rows of rhs
            kt_tiles = [
                cpool.tile([128, ST], BF16, name=f"kt{i}", tag=f"kt{i}")
                for i in range(3)
            ]
            for kt in kt_tiles:
                nc.gpsimd.memset(kt[H:128, :], 0.0)

            # per-stage state carried across pipeline iterations
            xt_sb = [None] * ns
            qk_sb = [None] * ns
            sm_sb = [None] * ns
            st_ps = [None] * ns
            v_ps = [None] * ns
            o_ps = [None] * ns
            o_pair = [None] * (ns // 2)

            for it in range(ns + 3):
                s = it          # projection stage
                s1 = it - 1     # scores stage
                s2 = it - 2     # output stage
                s3 = it - 3     # store stage

                if 0 <= s3 < ns and s3 % 2 == 1:
                    # 8. store 2 supertiles bf16, partition-major (1KB/part).
                    # Lagged so its dependency is complete: never stalls the
                    # sync prefetch queue.
                    nc.sync.dma_start(od[s3 // 2], o_pair[s3 // 2])

                if s < ns:
                    # 1. load xT (already transposed + bf16 on host); sync
                    # queue carries only cheap issues so it prefetches ahead
                    xt_sb[s] = xtpool.tile([128, NCH, ST], BF16,
                                           name="xt", tag="xt")
                    nc.sync.dma_start(xt_sb[s], xtd[s])

                    # 2. fused Q^T|K^T: one MM per chunk, 128 PE columns
                    qk_ps = pqk.tile([128, ST], F32, name="qkps", tag="qkps")
                    for j in range(NCH):
                        nc.tensor.matmul(
                            qk_ps,
                            lhsT=wqk[:, j, :],
                            rhs=xt_sb[s][:, j, :],
                            start=(j == 0),
                            stop=(j == NCH - 1),
                        )

                # 4/5/6 interleaved on PE: V(s) hides its weight loads
                # under the longer score/output streams of older stages
                for g in range(G):
                    if s < ns:
                        if g == 0:
                            v_ps[s] = pv.tile([128, G, H], F32,
                                              name="vps", tag="vps")
                        for j in range(NCH):
                            nc.tensor.matmul(
                                v_ps[s][:, g, :],
                                lhsT=xt_sb[s][:, j, g * 128:(g + 1) * 128],
                                rhs=wv[:, j, :],
                                start=(j == 0),
                                stop=(j == NCH - 1),
                            )
                    if 0 <= s1 < ns:
                        if g == 0:
                            st_ps[s1] = pst.tile([128, G, 128], F32,
                                                 name="stps", tag="stps")
                        # mask bias preload: st[s,q] = MB*mask[s,q]
                        nc.tensor.matmul(
                            st_ps[s1][:, g, :],
                            lhsT=masktt,
                            rhs=eyemb,
                            start=True,
                            stop=False,
                        )
                        nc.tensor.matmul(
                            st_ps[s1][:, g, :],
                            lhsT=kt_tiles[s1 % 3][:, g * 128:(g + 1) * 128],
                            rhs=qk_sb[s1][:, g * 128:(g + 1) * 128],
                            start=False,
                            stop=True,
                        )
                    if 0 <= s2 < ns:
                        if g == 0:
                            o_ps[s2] = po.tile([128, G, H + 1], F32,
                                               name="ops", tag="ops")
                        nc.tensor.matmul(
                            o_ps[s2][:, g, :],
                            lhsT=sm_sb[s2][:, g, :],
                            rhs=v_tiles[s2 % NV][:, g, :],
                            start=True,
                            stop=True,
                        )

                if s < ns:
                    # 2b. PSUM->SBUF, then 3. shuffle K^T down to base 0
                    qk_sb[s] = qkpool.tile([128, ST], BF16, name="qk", tag="qk")
                    nc.vector.tensor_copy(qk_sb[s], qk_ps)
                    nc.gpsimd.dma_start(
                        kt_tiles[s % 3][0:H, :], qk_sb[s][H:2 * H, :]
                    )

                if 0 <= s1 < ns:
                    # 5b. exp((scores + MB*mask)*SCALE - MB*SCALE): allowed
                    # entries exact, forbidden -> exp(-40.8) ~ 0
                    sm_sb[s1] = smpool.tile([128, G, 128], BF16,
                                            name="sm", tag="sm")
                    nc.scalar.activation(sm_sb[s1], st_ps[s1], AF.Exp,
                                         scale=SCALE, bias=ebias[:, 0:1])

                if s < ns:
                    # 4b. V PSUM->SBUF into the persistent [V|1] tile
                    nc.scalar.copy(v_tiles[s % NV][:, :, 0:H], v_ps[s])

                if 0 <= s2 < ns:
                    # 7. normalize (recip broadcast along heads), bf16 out
                    if s2 % 2 == 0:
                        o_pair[s2 // 2] = opool.tile([128, 2, G, H], BF16,
                                                     name="o", tag="o")
                    recip = opool.tile([128, G], F32, name="recip", tag="recip")
                    nc.vector.reciprocal(recip, o_ps[s2][:, :, H])
                    nc.vector.tensor_mul(
                        o_pair[s2 // 2][:, s2 % 2, :, :],
                        o_ps[s2][:, :, 0:H],
                        recip[:, :, None].to_broadcast([128, G, H]),
                    )

    nc.finalize()
    return nc


def _consts():
    bf = ml_dtypes.bfloat16
    maskt = np.kron(
        np.eye(128 // T, dtype=np.float32),
        np.triu(np.ones((T, T), dtype=np.float32)),
    )
    masktt = np.ascontiguousarray(maskt.T).astype(bf)
    eyemb = (MB * np.eye(128, dtype=np.float32)).astype(bf)
    return masktt, eyemb


def _prepare(x, Wq, Wk, Wv):
    """Returns (nc, in_maps) for the full-size problem."""
    assert x.shape == (B_FULL, T, C), x.shape
    ns = TOK // ST
    if ns not in _nc_cache:
        _nc_cache[ns] = _build_nc(ns)
    nc = _nc_cache[ns]

    bf = ml_dtypes.bfloat16
    wqk_full = np.concatenate([Wq, Wk], axis=1)  # [C, 2H]
    wqk_h = np.ascontiguousarray(
        wqk_full.reshape(NCH, 128, 2 * H).transpose(1, 0, 2)
    ).astype(bf)
    wv_h = np.ascontiguousarray(
        Wv.reshape(NCH, 128, H).transpose(1, 0, 2)
    ).astype(bf)
    masktt, eyemb = _consts()

    # host-side marshalling: bf16 cast + transpose to [ns, 128c, NCH, ST]
    xb = x.reshape(N_CORES, TOK // ST, ST, NCH, 128).astype(bf)
    in_maps = []
    for c in range(N_CORES):
        xs = np.ascontiguousarray(xb[c].transpose(0, 3, 2, 1))
        in_maps.append({
            "xt": xs, "wqk": wqk_h, "wv": wv_h,
            "masktt": masktt, "eyemb": eyemb,
        })
    return nc, in_maps


def _gather(results):
    ns = TOK // ST
    outs = [
        np.asarray(r["out"])
        .reshape(ns // 2, 128, 2, G, H)
        .transpose(0, 2, 3, 1, 4)
        .reshape(BP, T, H)
        .astype(np.float32)
        for r in results
    ]
    return np.concatenate(outs, axis=0)


def kernel(x, Wq, Wk, Wv):
    nc, in_maps = _prepare(x, Wq, Wk, Wv)
    res = run_bass_kernel_spmd(nc, in_maps, core_ids=list(range(N_CORES)))
    return _gather(res.results)


# revision 11
# speedup vs baseline: 1.2631x; 1.0530x over previous
"""Trainium2 Bass kernel for nn_Head (single-head causal attention, T=8).

Pure data parallel over 8 NeuronCores: per core x [4096, 8, 384] -> out
[4096, 8, 64]. The host marshals x into transposed bf16 layout
xT [ns, 128c, 3, 512tok] so the device streams contiguous tiles and never
transposes activations on-chip.

Per 512-token supertile (tokens on partitions in 4 groups of 128),
software-pipelined 3 deep on the PE (projections for s, scores for s-1,
output for s-2) so every PE operand is ready >= 1 iteration early and the
tensor engine never idles (keeps the 2.4 GHz p-state):
  1. DMA xT chunk [128, 3, 512] bf16 (contiguous)
  2. QK^T = [Wq|Wk]^T @ x^T -> qk [128(2h), 512] (3 fused MMs, W stationary)
  3. shuffle K^T (partitions 64:128) down to base 0 via SBUF->SBUF DMA
  4. V natural [tok, 64] = x @ Wv via lhsT=xT slices (12 MMs), interleaved
     with the s-1 score MMs and s-2 output MMs
  5. S^T[k,q] per group: PSUM preloaded with +800*mask via a matmul
     (lhsT=mask^T, rhs=800*I), then lhsT=K^T rhs=Q^T accumulates on top;
     exp(x*SCALE - 800*SCALE) on ACT zeroes forbidden entries -- no
     separate mask op on any vector engine
  6. out[q, 0:65] = S~ @ [V|1] via lhsT=S~^T (4 MMs); col 64 = softmax denom
  7. normalize: reciprocal + scaled copy (DVE), bf16 out
  8. DMA out bf16, partition-major, batched 2 supertiles per descriptor

bf16 on matmul paths with f32 PSUM accumulation: rel error ~3e-3.
"""

import numpy as np
import ml_dtypes

import concourse.bass as bass
import concourse.mybir as mybir
from concourse import bacc
from concourse.tile import TileContext
from concourse.bass_utils import run_bass_kernel_spmd

N_CORES = 8
B_FULL = 32768
T = 8
C = 384
H = 64

BP = B_FULL // N_CORES       # batch rows per core
TOK = BP * T                 # tokens per core
ST = 512                     # tokens per supertile
G = ST // 128                # 128-token groups per supertile
NCH = C // 128               # contraction chunks
SCALE = float(C) ** -0.5
NV = 4                       # persistent [V|1] buffers (pipeline depth 3)
MB = 800.0                   # mask bias magnitude (exact in bf16)

BF16 = mybir.dt.bfloat16
F32 = mybir.dt.float32
AF = mybir.ActivationFunctionType

_nc_cache = {}


def _build_nc(ns: int):
    """Build the Bass module for `ns` supertiles per core."""
    assert ns % 2 == 0
    nc = bacc.Bacc("TRN2", target_bir_lowering=False, debug=False)

    xtd = nc.dram_tensor("xt", [128, ns, NCH, ST], BF16, kind="ExternalInput")
    wqkd = nc.dram_tensor("wqk", [128, NCH, 2 * H], BF16, kind="ExternalInput")
    wvd = nc.dram_tensor("wv", [128, NCH, H], BF16, kind="ExternalInput")
    masktt_d = nc.dram_tensor("masktt", [128, 128], BF16, kind="ExternalInput")
    eye_d = nc.dram_tensor("eyemb", [128, 128], BF16, kind="ExternalInput")
    od = nc.dram_tensor("out", [ns // 2, 128, 2, G, H], BF16,
                        kind="ExternalOutput")

    with TileContext(nc) as tc:
        with (
            tc.tile_pool(name="const", bufs=1) as cpool,
            tc.tile_pool(name="xt", bufs=4) as xtpool,
            tc.tile_pool(name="qk", bufs=3) as qkpool,
            tc.tile_pool(name="sm", bufs=3) as smpool,
            tc.tile_pool(name="oo", bufs=3) as opool,
            tc.tile_pool(name="ps_qk", bufs=2, space="PSUM") as pqk,
            tc.tile_pool(name="ps_st", bufs=2, space="PSUM") as pst,
            tc.tile_pool(name="ps_v", bufs=2, space="PSUM") as pv,
            tc.tile_pool(name="ps_o", bufs=2, space="PSUM") as po,
        ):
            masktt = cpool.tile([128, 128], BF16)
            nc.sync.dma_start(masktt, masktt_d[:, :])
            eyemb = cpool.tile([128, 128], BF16)
            nc.sync.dma_start(eyemb, eye_d[:, :])
            wqk = cpool.tile([128, NCH, 2 * H], BF16)
            nc.sync.dma_start(wqk, wqkd[:, :, :])
            wv = cpool.tile([128, NCH, H], BF16)
            nc.sync.dma_start(wv, wvd[:, :, :])
            # persistent [V|1] tiles: ones column written once, V columns
            # rewritten every NV iterations (WAR tracked by the framework)
            v_tiles = [
                cpool.tile([128, G, H + 1], BF16, name=f"v{i}", tag=f"v{i}")
                for i in range(NV)
            ]
            for vt in v_tiles:
                nc.gpsimd.memset(vt[:, :, H:H + 1], 1.0)
            ebias = cpool.tile([128, 1], F32)
            nc.gpsimd.memset(ebias, -MB * SCALE)
            # persistent K^T pair tiles, full 128 partitions so the score MM
            # keeps the (128,128) PE tile config (a 64-row MM costs a ~125ns
            # reconfig); rows 64:128 stay zero to kill the K^T rows of rhs.
            # Paired (2 supertiles per shuffle) to halve DMA issue cost.
            kt_tiles = [
                cpool.tile([128, 2, ST], BF16, name=f"kt{i}", tag=f"kt{i}")
                for i in range(3)
            ]
            for kt in kt_tiles:
                nc.gpsimd.memset(kt[H:128, :, :], 0.0)

            # per-stage state carried across pipeline iterations
            xt_pair = [None] * (ns // 2)
            qk_pair = [None] * (ns // 2)
            sm_sb = [None] * ns
            st_ps = [None] * ns
            v_ps = [None] * ns
            o_ps = [None] * ns
            o_pair = [None] * (ns // 2)

            for it in range(ns + 4):
                s = it          # projection stage
                s1 = it - 2     # scores stage (lag 2: paired shuffle lands)
                s2 = it - 3     # output stage
                s3 = it - 4     # store stage

                if 0 <= s3 < ns and s3 % 2 == 1:
                    # 8. store 2 supertiles bf16, partition-major (1KB/part).
                    # Lagged so its dependency is complete: never stalls the
                    # sync prefetch queue.
                    nc.sync.dma_start(od[s3 // 2], o_pair[s3 // 2])

                if s < ns and s % 2 == 0:
                    # 1. load 2 supertiles of xT (partition-major dram: 6KB
                    # contiguous per partition); sync queue carries only
                    # cheap issues so it prefetches ahead
                    xt_pair[s // 2] = xtpool.tile([128, 2, NCH, ST], BF16,
                                                  name="xt", tag="xt")
                    nc.sync.dma_start(xt_pair[s // 2],
                                      xtd[:, s:s + 2])

                if s < ns:
                    xt_sb = xt_pair[s // 2][:, s % 2]
                    # 2. fused Q^T|K^T: one MM per chunk, 128 PE columns
                    qk_ps = pqk.tile([128, ST], F32, name="qkps", tag="qkps")
                    for j in range(NCH):
                        nc.tensor.matmul(
                            qk_ps,
                            lhsT=wqk[:, j, :],
                            rhs=xt_sb[:, j, :],
                            start=(j == 0),
                            stop=(j == NCH - 1),
                        )

                # 4/5/6 interleaved on PE: V(s) hides its weight loads
                # under the longer score/output streams of older stages
                for g in range(G):
                    if s < ns:
                        if g == 0:
                            v_ps[s] = pv.tile([128, G, H], F32,
                                              name="vps", tag="vps")
                        for j in range(NCH):
                            nc.tensor.matmul(
                                v_ps[s][:, g, :],
                                lhsT=xt_sb[:, j, g * 128:(g + 1) * 128],
                                rhs=wv[:, j, :],
                                start=(j == 0),
                                stop=(j == NCH - 1),
                            )
                    if 0 <= s1 < ns:
                        if g == 0:
                            st_ps[s1] = pst.tile([128, G, 128], F32,
                                                 name="stps", tag="stps")
                        # mask bias preload: st[s,q] = MB*mask[s,q]
                        nc.tensor.matmul(
                            st_ps[s1][:, g, :],
                            lhsT=masktt,
                            rhs=eyemb,
                            start=True,
                            stop=False,
                        )
                        nc.tensor.matmul(
                            st_ps[s1][:, g, :],
                            lhsT=kt_tiles[(s1 // 2) % 3][
                                :, s1 % 2, g * 128:(g + 1) * 128],
                            rhs=qk_pair[s1 // 2][
                                :, s1 % 2, g * 128:(g + 1) * 128],
                            start=False,
                            stop=True,
                        )
                    if 0 <= s2 < ns:
                        if g == 0:
                            o_ps[s2] = po.tile([128, G, H + 1], F32,
                                               name="ops", tag="ops")
                        nc.tensor.matmul(
                            o_ps[s2][:, g, :],
                            lhsT=sm_sb[s2][:, g, :],
                            rhs=v_tiles[s2 % NV][:, g, :],
                            start=True,
                            stop=True,
                        )

                if s < ns:
                    # 2b. PSUM->SBUF, then 3. shuffle K^T down to base 0
                    # (one SBUF->SBUF DMA per 2 supertiles: 2KB/partition)
                    if s % 2 == 0:
                        qk_pair[s // 2] = qkpool.tile([128, 2, ST], BF16,
                                                      name="qk", tag="qk")
                    nc.vector.tensor_copy(qk_pair[s // 2][:, s % 2], qk_ps)
                    if s % 2 == 1:
                        nc.gpsimd.dma_start(
                            kt_tiles[(s // 2) % 3][0:H, :, :],
                            qk_pair[s // 2][H:2 * H, :, :],
                        )

                if 0 <= s1 < ns:
                    # 5b. exp((scores + MB*mask)*SCALE - MB*SCALE): allowed
                    # entries exact, forbidden -> exp(-40.8) ~ 0
                    sm_sb[s1] = smpool.tile([128, G, 128], BF16,
                                            name="sm", tag="sm")
                    nc.scalar.activation(sm_sb[s1], st_ps[s1], AF.Exp,
                                         scale=SCALE, bias=ebias[:, 0:1])

                if s < ns:
                    # 4b. V PSUM->SBUF into the persistent [V|1] tile
                    nc.scalar.copy(v_tiles[s % NV][:, :, 0:H], v_ps[s])

                if 0 <= s2 < ns:
                    # 7. normalize (recip broadcast along heads), bf16 out
                    if s2 % 2 == 0:
                        o_pair[s2 // 2] = opool.tile([128, 2, G, H], BF16,
                                                     name="o", tag="o")
                    recip = opool.tile([128, G], F32, name="recip", tag="recip")
                    nc.vector.reciprocal(recip, o_ps[s2][:, :, H])
                    nc.vector.tensor_mul(
                        o_pair[s2 // 2][:, s2 % 2, :, :],
                        o_ps[s2][:, :, 0:H],
                        recip[:, :, None].to_broadcast([128, G, H]),
                    )

    nc.finalize()
    return nc


def _consts():
    bf = ml_dtypes.bfloat16
    maskt = np.kron(
        np.eye(128 // T, dtype=np.float32),
        np.triu(np.ones((T, T), dtype=np.float32)),
    )
    masktt = np.ascontiguousarray(maskt.T).astype(bf)
    eyemb = (MB * np.eye(128, dtype=np.float32)).astype(bf)
    return masktt, eyemb


def _prepare(x, Wq, Wk, Wv):
    """Returns (nc, in_maps) for the full-size problem."""
    assert x.shape == (B_FULL, T, C), x.shape
    ns = TOK // ST
    if ns not in _nc_cache:
        _nc_cache[ns] = _build_nc(ns)
    nc = _nc_cache[ns]

    bf = ml_dtypes.bfloat16
    wqk_full = np.concatenate([Wq, Wk], axis=1)  # [C, 2H]
    wqk_h = np.ascontiguousarray(
        wqk_full.reshape(NCH, 128, 2 * H).transpose(1, 0, 2)
    ).astype(bf)
    wv_h = np.ascontiguousarray(
        Wv.reshape(NCH, 128, H).transpose(1, 0, 2)
    ).astype(bf)
    masktt, eyemb = _consts()

    # host-side marshalling: bf16 cast + transpose to [128c, ns, NCH, ST]
    xb = x.reshape(N_CORES, TOK // ST, ST, NCH, 128).astype(bf)
    in_maps = []
    for c in range(N_CORES):
        xs = np.ascontiguousarray(xb[c].transpose(3, 0, 2, 1))
        in_maps.append({
            "xt": xs, "wqk": wqk_h, "wv": wv_h,
            "masktt": masktt, "eyemb": eyemb,
        })
    return nc, in_maps


def _gather(results):
    ns = TOK // ST
    outs = [
        np.asarray(r["out"])
        .reshape(ns // 2, 128, 2, G, H)
        .transpose(0, 2, 3, 1, 4)
        .reshape(BP, T, H)
        .astype(np.float32)
        for r in results
    ]
    return np.concatenate(outs, axis=0)


def kernel(x, Wq, Wk, Wv):
    nc, in_maps = _prepare(x, Wq, Wk, Wv)
    res = run_bass_kernel_spmd(nc, in_maps, core_ids=list(range(N_CORES)))
    return _gather(res.results)
